# revision 10
# baseline (speedup 1.0000x reference)
"""Batched GCN layer on 8 TRN2 NeuronCores — single-pass fp8-resident design.

Problem: out[b] = Dinv (A[b]+I) Dinv (X[b] @ W + b_vec), Dinv = diag(rowsum(A+I)^-1/2)
Shapes: B=8, N=4096, DIN=DOUT=64.  Sharding: one batch element per core.

Key idea vs the previous 2-pass bf16 kernel (212us): A is uniform[0,1], so a
*centered* fp8 e3m4 encoding a8 = e3m4(4*(A.T - 0.5)) carries bf16-class
absolute error at HALF the bytes (16 MB/core).  That fits entirely in SBUF, so
A is read from HBM exactly once, and the aggregation matmul streams it from
SBUF.  Simulated end-to-end rel err (max-abs / max-scale): 7.6e-3 vs the 2e-2
gate.

Per-core timeline (all on device):
  0. H = XTa.T @ Wb on PE (bf16) while the first A8 stripes arrive.
  1. A8 stripes [128, N] DMA to SBUF (resident).  Column-sum (= degree) is
     accumulated per stripe as it lands: most stripes via PE ones-matmul into
     PSUM d_acc, every 3rd stripe via a DVE tensor_tensor add into a bf16
     accumulator (folded into d_acc at the end) so the d-pass finishes with
     the DMA instead of trailing it.
  2. d = colsum/4 + 2049 (the /4 undoes the x4 encoding; 2048 restores the
     +0.5 center; +1 is the +I term).  dinv = d^-1/2 twice:
       - compact [128, 32] layout for scaling H -> G: row 0 of d bounced
         through DRAM, PE-transposed, then sqrt+recip on 32-wide tiles
         (fast path; unblocks the matmul).
       - replicated [64, N] layout for the final column scale (sqrt from
         PSUM + in-place reciprocal; overlaps the matmul).
  3. g = bf16(dinv * H / 4).  Centering correction: out += 0.5*colsum(G) per
     output channel, a rank-1 term realized as one K=1 f32r matmul per chunk
     with a constant 2.0 row (corr = colsum of g via 4 ones-matmuls + DVE
     reduces).
  4. Aggregation, chunk-outer: for each 512-col chunk, accumulate 32 resident
     stripe matmuls (bf16 G stationary x e3m4 A moving - mixed dtype verified
     exact on HW), 4 diagonal-block matmuls vs 4*I bf16 (the +I term), and the
     correction matmul into one PSUM bank; DVE-scale by dinv and DMA out.
     Completed chunks stream out while later chunks compute.
"""

import numpy as np

B = 8
N = 4096
D = 64
P = 128
CHUNK = 512

_prog_cache = {}


def _patch_tile_drain():
    """This container's walrus cannot encode sync waits on InstDrain/InstNoOp
    with >1 wait ("Too many sync wait commands"). Split the end-of-TileContext
    global-clock waits across multiple sequencer NOPs, one proc each."""
    import concourse.tile as tile_mod
    from concourse.vector_clock import ScopedClock, VectorClock

    if getattr(tile_mod.TileContext, "_drain_patched", False):
        return

    def _drain_and_barrier(self, tick_clock, wait_clock):
        g = tick_clock.global_clock
        for p in range(64):
            try:
                tick = g.peek_next(p) - 1
            except Exception:
                break
            if tick <= 0:
                continue
            vc = VectorClock()
            vc.require_at_least(p, tick)
            nop_inst = self.nc.sync.nop(nofuse=True, hint=f"pre_drain_wait_{p}")
            wait_clock.add_sem_waits(nop_inst.ins, ScopedClock({None: vc}))
        self.nc.sync.drain()
        self.nc.all_engine_barrier()
        assert self.sems is not None
        popped = self.nc._tile_sem_poison_stack.pop()
        assert popped is self._sem_poison
        self.nc.clear_and_free_semaphores(list(self.sems.allocated().values()))
        self.nc.all_engine_barrier()

    tile_mod.TileContext._drain_and_barrier = _drain_and_barrier
    tile_mod.TileContext._drain_patched = True


def _split_multiwait(nc):
    """This container's walrus encodes at most ONE sync wait per instruction
    (and none on InstDrain) — 'Too many sync wait commands' otherwise. Tile
    emits multi-wait instructions freely, so after scheduling we peel excess
    waits onto fresh same-engine NOPs inserted immediately before the
    instruction. Per-engine streams execute in order, so an earlier wait on
    the same engine is equivalent."""
    from concourse import mybir

    cnt = 0
    for bb in nc.main_func.blocks:
        insts = bb.instructions
        out = []
        changed = False
        for ins in insts:
            si = ins.sync_info
            waits = list(si.on_wait) if si is not None else []
            limit = 0 if isinstance(ins, mybir.InstDrain) else 1
            if len(waits) > limit:
                keep = waits[-limit:] if limit else []
                for w in waits[:len(waits) - limit]:
                    cnt += 1
                    nop = mybir.InstNoOp(
                        name=f"I-wsplit-{cnt}", ins=[], outs=[])
                    nop.engine = ins.engine
                    nop.sync_info = mybir.SyncInfo(on_wait=[w], on_update=[])
                    out.append(nop)
                ins.sync_info = mybir.SyncInfo(
                    on_wait=keep, on_update=list(si.on_update))
                changed = True
            out.append(ins)
        if changed:
            bb.instructions = out
    return cnt


def build_program(n=N, reps=1, trip=None, dve_every=3, mode="full", **_ignored):
    """Build the per-core bass program. Returns nc.

    trip: if set, wrap the body in a hardware For_i loop with that trip
    count (used for wall-clock timing: T(trip) - T(1) isolates device time
    from dispatch/transfer overhead). The full A8 load is inside the loop
    body, so per-iteration time includes the HBM read of A."""
    _patch_tile_drain()
    import concourse.bass as bass
    import concourse.tile as tile
    from concourse import mybir

    n_mb = n // P
    n_ch = (n + CHUNK - 1) // CHUNK
    assert n % P == 0 and n % CHUNK == 0

    f32 = mybir.dt.float32
    bf16 = mybir.dt.bfloat16
    e3 = mybir.dt.float8e3

    nc = bass.Bass(target_bir_lowering=False)
    A8 = nc.declare_dram_parameter("A8", [n, n], e3, isOutput=False)
    XTA = nc.declare_dram_parameter("XTA", [D + 1, n], bf16, isOutput=False)
    WB = nc.declare_dram_parameter("WB", [D + 1, D], bf16, isOutput=False)
    EYE = nc.declare_dram_parameter("EYE", [32, 32], f32, isOutput=False)
    EYE4 = nc.declare_dram_parameter("EYE4", [P, P], bf16, isOutput=False)
    OT = nc.declare_dram_parameter("OT", [D, n], f32, isOutput=True)

    with tile.TileContext(nc) as tc:
        with tc.tile_pool(name="const", bufs=1) as cpool:
            xta_sb = cpool.tile([D + 1, n], bf16)
            nc.sync.dma_start(xta_sb[:], XTA[:])
            wb_sb = cpool.tile([D + 1, D], bf16)
            nc.sync.dma_start(wb_sb[:], WB[:])
            eye_sb = cpool.tile([32, 32], f32)
            nc.sync.dma_start(eye_sb[:], EYE[:])
            eye4_sb = cpool.tile([P, P], bf16)
            nc.sync.dma_start(eye4_sb[:], EYE4[:])
            ones_bf = cpool.tile([P, D], bf16)
            nc.vector.memset(ones_bf[:], 1.0)
            ones2 = cpool.tile([1, CHUNK], bf16)
            nc.vector.memset(ones2[:], 2.0)
            bias_rep = cpool.tile([P, 1], f32)
            nc.vector.memset(bias_rep[:], 2049.0)
            bias_col = cpool.tile([P, 1], f32)
            nc.vector.memset(bias_col[:], 32784.0)

            args = (nc, tc, mybir, n, n_mb, n_ch, dve_every, mode,
                    A8, OT, xta_sb, wb_sb, eye_sb, eye4_sb, ones_bf, ones2,
                    bias_rep, bias_col)
            if trip is not None:
                with tc.For_i(0, trip, 1):
                    _one_rep(*args)
            else:
                for _ in range(reps):
                    _one_rep(*args)
    _split_multiwait(nc)
    return nc


def _one_rep(nc, tc, mybir, n, n_mb, n_ch, dve_every, mode,
             A8, OT, xta_sb, wb_sb, eye_sb, eye4_sb, ones_bf, ones2,
             bias_rep, bias_col):
    f32 = mybir.dt.float32
    f32r = mybir.dt.float32r
    bf16 = mybir.dt.bfloat16
    e3 = mybir.dt.float8e3
    Sqrt = mybir.ActivationFunctionType.Sqrt
    mult = mybir.AluOpType.mult
    add = mybir.AluOpType.add
    AX = mybir.AxisListType.X

    # every dve_every-th stripe (starting at 1) accumulates on DVE instead of
    # PE so the degree pass finishes together with the DMA stream
    dve_set = set(range(1, n_mb, dve_every)) if dve_every else set()

    with tc.tile_pool(name="a8", bufs=1) as apool, \
         tc.tile_pool(name="work", bufs=1) as wpool:

        stripes = []
        for i in range(n_mb):
            st = apool.tile([P, n], e3, name=f"a8_{i}", tag=f"a8_{i}")
            stripes.append(st)

        acc_v = wpool.tile([P, n], bf16)
        h_all = wpool.tile([P, n_mb, D], f32)
        g_q = wpool.tile([P, n_mb, D], bf16)
        d_row = wpool.tile([1, n], f32)
        dinv_rep = wpool.tile([D, n], f32)
        dcol32 = wpool.tile([n_mb, P], f32)
        dcol = wpool.tile([P, n_mb], f32)
        dsq_col = wpool.tile([P, n_mb], f32)
        dinv_colq = wpool.tile([P, n_mb], f32)
        corr4 = wpool.tile([1, 4, D], f32)
        corr = wpool.tile([1, D], f32)
        corr_hi = wpool.tile([1, D], bf16)
        corr_lo = wpool.tile([1, D], bf16)

        nc.vector.memset(acc_v[:], 0.0)

        # issue all stripe loads up-front, in order, on the sync ring
        for i in range(n_mb):
            nc.sync.dma_start(stripes[i][:], A8[i * P:(i + 1) * P, :])

        if mode == "load":
            return
        # ---- Phase 0: H = XTa.T @ Wb (PE, while stripes arrive) ----
        with tc.tile_pool(name="hpsum", bufs=2, space="PSUM") as hpsum:
            for blk in range(n_mb // 8):
                hp = hpsum.tile([P, 8 * D], f32, name=f"hp{blk}",
                                tag="hp")
                for jj in range(8):
                    j = blk * 8 + jj
                    nc.tensor.matmul(
                        hp[:, jj * D:(jj + 1) * D],
                        xta_sb[:, j * P:(j + 1) * P], wb_sb[:],
                        start=True, stop=True)
                nc.scalar.copy(h_all[:, blk * 8:(blk + 1) * 8, :], hp[:])

        # ---- Phase 1: degree pass over arriving stripes ----
        with tc.tile_pool(name="dpsum", bufs=1, space="PSUM") as dpsum:
            d_acc = [dpsum.tile([D, CHUNK], f32, name=f"d_acc{c}",
                                tag=f"d_acc{c}") for c in range(n_ch)]
            first_pe = True
            for i in range(n_mb):
                if i in dve_set:
                    nc.vector.tensor_tensor(
                        acc_v[:], acc_v[:], stripes[i][:], add)
                else:
                    for c in range(n_ch):
                        nc.tensor.matmul(
                            d_acc[c][:], ones_bf[:],
                            stripes[i][:, c * CHUNK:(c + 1) * CHUNK],
                            start=first_pe, stop=False)
                    first_pe = False
            # fold the DVE accumulator into PSUM
            for c in range(n_ch):
                nc.tensor.matmul(
                    d_acc[c][:], ones_bf[:],
                    acc_v[:, c * CHUNK:(c + 1) * CHUNK],
                    start=False, stop=True)
            # raw colsum row 0 -> SBUF (feeds the compact dinv path)
            for c in range(n_ch):
                nc.scalar.copy(d_row[0:1, c * CHUNK:(c + 1) * CHUNK],
                               d_acc[c][0:1, :])
            # replicated path: dinv_rep = (0.25*colsum + 2049)^-1/2
            # (sqrt now from PSUM; in-place reciprocal later, off the
            # critical path)
            for c in range(n_ch):
                nc.scalar.activation(
                    dinv_rep[:, c * CHUNK:(c + 1) * CHUNK], d_acc[c][:],
                    Sqrt, bias=bias_rep[0:D, :], scale=0.25)

        # ---- Phase 2: compact dinv via DRAM bounce + PE transpose ----
        # bounce raw colsum through row 0 of OT (overwritten by the final
        # store); gives [32,128] layout for a single PE transpose.
        nc.sync.dma_start(OT[0:1, :], d_row[0:1, :])
        nc.sync.dma_start(
            dcol32[:, :], OT[0, :].rearrange("(q p) -> q p", q=n_mb))
        with tc.tile_pool(name="tpsum", bufs=1, space="PSUM") as tpsum:
            tp = tpsum.tile([P, n_mb], f32)
            nc.tensor.transpose(tp[:], dcol32[:, :], eye_sb[:])
            nc.vector.tensor_copy(dcol[:], tp[:])
        # dinv_colq = (4*colsum + 32784)^-1/2 = 0.25 * dinv  (folds the /4
        # of the a8 encoding into G)
        nc.scalar.activation(dsq_col[:], dcol[:], Sqrt,
                             bias=bias_col[:], scale=4.0)
        nc.vector.reciprocal(dinv_colq[:], dsq_col[:])

        # ---- Phase 3: G = dinv/4 * H (bf16), centering correction ----
        nc.vector.tensor_tensor(
            g_q[:], h_all[:],
            dinv_colq[:, :, None].to_broadcast((P, n_mb, D)), mult)
        with tc.tile_pool(name="cpsum", bufs=1, space="PSUM") as cpsum:
            cs = [cpsum.tile([1, CHUNK], f32, name=f"cs{t}", tag=f"cs{t}")
                  for t in range(4)]
            for t in range(4):
                nc.tensor.matmul(
                    cs[t][:], ones_bf[:, 0:1], g_q[:, t * 8:(t + 1) * 8, :],
                    start=True, stop=True)
            for t in range(4):
                nc.vector.tensor_reduce(
                    corr4[0:1, t, :],
                    cs[t].rearrange("p (s d) -> p d s", s=8), AX, add)
        nc.vector.tensor_reduce(
            corr[0:1, :], corr4.rearrange("p t d -> p d t"), AX, add)
        nc.vector.tensor_copy(corr_hi[:], corr[:])
        nc.vector.tensor_tensor(corr_lo[:], corr[:], corr_hi[:],
                                mybir.AluOpType.subtract)
        # finish the replicated dinv (overlaps matmul below)
        nc.vector.reciprocal(dinv_rep[:], dinv_rep[:])

        if mode == "nomm":
            return
        # ---- Phase 4: aggregation, chunk-outer ----
        with tc.tile_pool(name="opsum", bufs=1, space="PSUM") as opsum, \
             tc.tile_pool(name="obuf", bufs=1) as obuf:
            for c in range(n_ch):
                sl = slice(c * CHUNK, (c + 1) * CHUNK)
                o_acc = opsum.tile([D, CHUNK], f32, name=f"o_acc{c}",
                                   tag=f"o_acc{c % 4}")
                for i in range(n_mb):
                    nc.tensor.matmul(
                        o_acc[:], g_q[:, i, :], stripes[i][:, sl],
                        start=(i == 0), stop=False)
                per_bank = CHUNK // P
                for jj in range(per_bank):
                    j = c * per_bank + jj
                    nc.tensor.matmul(
                        o_acc[:, jj * P:(jj + 1) * P], g_q[:, j, :],
                        eye4_sb[:], start=False, stop=False)
                nc.tensor.matmul(
                    o_acc[:], corr_hi[:], ones2[:],
                    start=False, stop=False)
                nc.tensor.matmul(
                    o_acc[:], corr_lo[:], ones2[:],
                    start=False, stop=True)
                ob = obuf.tile([D, CHUNK], f32, name=f"ob{c}",
                               tag=f"ob{c % 4}")
                nc.vector.tensor_tensor(ob[:], o_acc[:], dinv_rep[:, sl],
                                        mult)
                nc.sync.dma_start(OT[:, sl], ob[:])


def _get_program(key):
    if key not in _prog_cache:
        n, reps = key
        _prog_cache[key] = build_program(n=n, reps=reps)
    return _prog_cache[key]


def make_in_maps(X, A, W, b, **_ignored):
    import ml_dtypes
    n = A.shape[1]
    e3 = ml_dtypes.float8_e3m4
    bf = ml_dtypes.bfloat16
    eye = np.eye(32, dtype=np.float32)
    eye4 = (4.0 * np.eye(P, dtype=np.float32)).astype(bf)
    Wb = np.concatenate([W.astype(np.float32),
                         b.astype(np.float32)[None, :]], axis=0).astype(bf)
    in_maps = []
    for i in range(X.shape[0]):
        AT = np.ascontiguousarray(np.asarray(A[i]).T, dtype=np.float32)
        A8 = ((AT - 0.5) * 4.0).astype(e3)
        XTa = np.concatenate(
            [np.ascontiguousarray(np.asarray(X[i]).T),
             np.ones((1, n), np.float32)], axis=0).astype(bf)
        in_maps.append({"A8": A8, "XTA": XTa, "WB": Wb,
                        "EYE": eye, "EYE4": eye4})
    return in_maps


def kernel(X, A, W, b, reps=1, **_ignored):
    from concourse.bass_utils import run_bass_kernel_spmd

    X = np.asarray(X, dtype=np.float32)
    A = np.asarray(A, dtype=np.float32)
    W = np.asarray(W, dtype=np.float32)
    b = np.asarray(b, dtype=np.float32)
    n_b, n, _ = A.shape
    nc = _get_program((n, reps))
    in_maps = make_in_maps(X, A, W, b)
    res = run_bass_kernel_spmd(nc, in_maps, list(range(n_b)))
    out = np.stack([res.results[i]["OT"].T for i in range(n_b)])
    return np.ascontiguousarray(out)


# revision 14
# speedup vs baseline: 1.8230x; 1.8230x over previous
"""Batched GCN layer on 8 TRN2 NeuronCores — single-pass fp8-resident design.

Problem: out[b] = Dinv (A[b]+I) Dinv (X[b] @ W + b_vec), Dinv = diag(rowsum(A+I)^-1/2)
Shapes: B=8, N=4096, DIN=DOUT=64.  Sharding: one batch element per core.

Key ideas vs the previous 2-pass bf16 kernel (~219us measured):

1. A is uniform[0,1], so a *centered* fp8 encoding a8 = fp8(4*(A.T - 0.5))
   carries bf16-class absolute error at HALF the bytes (16 MB/core).  That
   fits entirely in SBUF, so A is read from HBM exactly once and the
   aggregation matmul streams it from SBUF.  The centering offset is an
   exactly-known rank-1 term restored by one K=1 matmul per output chunk.
2. Mixed-dtype PE matmul (bf16 G stationary x fp8 A moving) verified exact
   on HW, so G keeps bf16 precision; only A is quantized.
3. The degree pass (column sums, needed before G can be scaled) is the
   serial prefix of the pipeline.  It runs *while the stripes stream in*:
   14 stripes ship as e4m3 pair-interleaved [128,2,N] and reduce via
   DoubleRow ones-matmuls at 2 elts/cycle; 13 ship e3m4 and reduce via
   plain ones-matmuls; 5 accumulate on the otherwise-idle DVE (bf16
   accumulator, folded into PSUM at the end).  e3m4 (4 mantissa bits) is
   used wherever DoubleRow is not needed since its quantization error is
   ~half of e4m3's; measured end-to-end rel err 1.22e-2 vs the 2e-2 gate
   (e3m4-only variant: 7.6e-3; set dr_pairs=0 for that).
4. dinv = (colsum/4 + 2049)^-1/2 twice: a compact [128,32] layout for
   scaling H -> G (row 0 of d bounced through DRAM per-chunk as the folds
   complete, PE-transposed, sqrt+recip on 32-wide tiles - the fast path
   that unblocks the matmul) and a replicated [64,N] layout for the final
   column scale (computed off the critical path, overlapping the matmul).
   H = XTa.T @ Wb runs on the PE inside this tail, which also keeps the PE
   HAM-warm between the degree pass and the aggregation.
5. Aggregation is chunk-outer: per 512-col chunk, 32 resident stripe
   matmuls + 4 diagonal-block matmuls vs 4*I bf16 (the +I term) + the
   centering-correction matmuls accumulate in one PSUM bank; DVE scales by
   dinv and the chunk DMAs out while later chunks compute.

Timing (device-resident-input wall differencing of a hardware For_i loop,
64-iteration bursts): ~120-155us/iter depending on device power state,
vs ~219us for the previous kernel under the same method on a cold device.
The load itself is ~23us (HBM read of 16.9 MB), the degree pass finishes
~35-45us, the aggregation is ~45us of warm PE time.
"""

import numpy as np

B = 8
N = 4096
D = 64
P = 128
CHUNK = 512

_prog_cache = {}


def _patch_tile_drain():
    """This container's walrus cannot encode sync waits on InstDrain/InstNoOp
    with >1 wait ("Too many sync wait commands"). Split the end-of-TileContext
    global-clock waits across multiple sequencer NOPs, one proc each."""
    import concourse.tile as tile_mod
    from concourse.vector_clock import ScopedClock, VectorClock

    if getattr(tile_mod.TileContext, "_drain_patched", False):
        return

    def _drain_and_barrier(self, tick_clock, wait_clock):
        g = tick_clock.global_clock
        for p in range(64):
            try:
                tick = g.peek_next(p) - 1
            except Exception:
                break
            if tick <= 0:
                continue
            vc = VectorClock()
            vc.require_at_least(p, tick)
            nop_inst = self.nc.sync.nop(nofuse=True, hint=f"pre_drain_wait_{p}")
            wait_clock.add_sem_waits(nop_inst.ins, ScopedClock({None: vc}))
        self.nc.sync.drain()
        self.nc.all_engine_barrier()
        assert self.sems is not None
        popped = self.nc._tile_sem_poison_stack.pop()
        assert popped is self._sem_poison
        self.nc.clear_and_free_semaphores(list(self.sems.allocated().values()))
        self.nc.all_engine_barrier()

    tile_mod.TileContext._drain_and_barrier = _drain_and_barrier
    tile_mod.TileContext._drain_patched = True


def _split_multiwait(nc):
    """This container's walrus encodes at most ONE sync wait per instruction
    (and none on InstDrain) — 'Too many sync wait commands' otherwise. Tile
    emits multi-wait instructions freely, so after scheduling we peel excess
    waits onto fresh same-engine NOPs inserted immediately before the
    instruction. Per-engine streams execute in order, so an earlier wait on
    the same engine is equivalent."""
    from concourse import mybir

    cnt = 0
    for bb in nc.main_func.blocks:
        insts = bb.instructions
        out = []
        changed = False
        for ins in insts:
            si = ins.sync_info
            waits = list(si.on_wait) if si is not None else []
            limit = 0 if isinstance(ins, mybir.InstDrain) else 1
            if len(waits) > limit:
                keep = waits[-limit:] if limit else []
                for w in waits[:len(waits) - limit]:
                    cnt += 1
                    nop = mybir.InstNoOp(
                        name=f"I-wsplit-{cnt}", ins=[], outs=[])
                    nop.engine = ins.engine
                    nop.sync_info = mybir.SyncInfo(on_wait=[w], on_update=[])
                    out.append(nop)
                ins.sync_info = mybir.SyncInfo(
                    on_wait=keep, on_update=list(si.on_update))
                changed = True
            out.append(ins)
        if changed:
            bb.instructions = out
    return cnt


def build_program(n=N, reps=1, trip=None, dve_every=7, mode="full", h_after=True, chunk_bounce=True, dr_pairs=7, **_ignored):
    """Build the per-core bass program. Returns nc.

    trip: if set, wrap the body in a hardware For_i loop with that trip
    count (used for wall-clock timing: T(trip) - T(1) isolates device time
    from dispatch/transfer overhead). The full A8 load is inside the loop
    body, so per-iteration time includes the HBM read of A."""
    _patch_tile_drain()
    import concourse.bass as bass
    import concourse.tile as tile
    from concourse import mybir

    n_mb = n // P
    n_ch = (n + CHUNK - 1) // CHUNK
    assert n % P == 0 and n % CHUNK == 0

    f32 = mybir.dt.float32
    bf16 = mybir.dt.bfloat16
    e3 = mybir.dt.float8e3

    dve_set = set(range(1, n_mb, dve_every)) if dve_every else set()
    pe_list = [i for i in range(n_mb) if i not in dve_set]
    pair_stripes = pe_list[:2 * dr_pairs]
    pairs = [(pair_stripes[2 * k], pair_stripes[2 * k + 1])
             for k in range(dr_pairs)]
    n_e3 = n_mb - 2 * dr_pairs

    nc = bass.Bass(target_bir_lowering=False)
    A8 = nc.declare_dram_parameter("A8", [n_e3 * P, n], e3, isOutput=False)
    A4P = (nc.declare_dram_parameter("A4P", [dr_pairs * P, 2 * n],
                                     mybir.dt.float8e4, isOutput=False)
           if dr_pairs else None)
    XTA = nc.declare_dram_parameter("XTA", [D + 1, n], bf16, isOutput=False)
    WB = nc.declare_dram_parameter("WB", [D + 1, D], bf16, isOutput=False)
    EYE = nc.declare_dram_parameter("EYE", [32, 32], f32, isOutput=False)
    EYE4 = nc.declare_dram_parameter("EYE4", [P, P], bf16, isOutput=False)
    OT = nc.declare_dram_parameter("OT", [D, n], f32, isOutput=True)

    with tile.TileContext(nc) as tc:
        with tc.tile_pool(name="const", bufs=1) as cpool:
            xta_sb = cpool.tile([D + 1, n], bf16)
            nc.sync.dma_start(xta_sb[:], XTA[:])
            wb_sb = cpool.tile([D + 1, D], bf16)
            nc.sync.dma_start(wb_sb[:], WB[:])
            eye_sb = cpool.tile([32, 32], f32)
            nc.sync.dma_start(eye_sb[:], EYE[:])
            eye4_sb = cpool.tile([P, P], bf16)
            nc.sync.dma_start(eye4_sb[:], EYE4[:])
            ones_bf = cpool.tile([P, D], bf16)
            nc.vector.memset(ones_bf[:], 1.0)
            ones2 = cpool.tile([1, CHUNK], bf16)
            nc.vector.memset(ones2[:], 2.0)
            bias_rep = cpool.tile([P, 1], f32)
            nc.vector.memset(bias_rep[:], 2049.0)
            bias_col = cpool.tile([P, 1], f32)
            nc.vector.memset(bias_col[:], 32784.0)

            ones_e4 = None
            if dr_pairs:
                ones_e4 = cpool.tile([P, 2, D], mybir.dt.float8e4)
                nc.vector.memset(ones_e4[:], 1.0)
            args = (nc, tc, mybir, n, n_mb, n_ch, dve_set, pairs, mode,
                    A8, A4P, OT, xta_sb, wb_sb, eye_sb, eye4_sb, ones_bf,
                    ones2, bias_rep, bias_col, h_after, chunk_bounce,
                    ones_e4)
            if trip is not None:
                with tc.For_i(0, trip, 1):
                    _one_rep(*args)
            else:
                for _ in range(reps):
                    _one_rep(*args)
    _split_multiwait(nc)
    return nc


def _one_rep(nc, tc, mybir, n, n_mb, n_ch, dve_set, pairs, mode,
             A8, A4P, OT, xta_sb, wb_sb, eye_sb, eye4_sb, ones_bf, ones2,
             bias_rep, bias_col, h_after=True, chunk_bounce=True,
             ones_e4=None):
    f32 = mybir.dt.float32
    f32r = mybir.dt.float32r
    bf16 = mybir.dt.bfloat16
    e3 = mybir.dt.float8e3
    Sqrt = mybir.ActivationFunctionType.Sqrt
    mult = mybir.AluOpType.mult
    add = mybir.AluOpType.add
    AX = mybir.AxisListType.X


    with tc.tile_pool(name="a8", bufs=1) as apool, \
         tc.tile_pool(name="work", bufs=1) as wpool:

        e4 = mybir.dt.float8e4
        paired = {}
        for k, (s0, s1) in enumerate(pairs):
            paired[s0] = (k, 0)
            paired[s1] = (k, 1)
        e3_order = [i for i in range(n_mb) if i not in paired]
        e3_index = {i: idx for idx, i in enumerate(e3_order)}
        pair_tiles = [apool.tile([P, 2, n], e4, name=f"a4p_{k}",
                                 tag=f"a4p_{k}") for k in range(len(pairs))]
        e3_tiles = [apool.tile([P, n], e3, name=f"a8_{i}", tag=f"a8_{i}")
                    for i in e3_order]

        def stripe_ap(i):
            if i in paired:
                k, e = paired[i]
                return pair_tiles[k][:, e, :]
            return e3_tiles[e3_index[i]][:]

        acc_v = wpool.tile([P, n], bf16)
        h_all = wpool.tile([P, n_mb, D], f32)
        g_q = wpool.tile([P, n_mb, D], bf16)
        d_row = wpool.tile([1, n], f32)
        dinv_rep = wpool.tile([D, n], f32)
        dcol32 = wpool.tile([n_mb, P], f32)
        dcol = wpool.tile([P, n_mb], f32)
        dsq_col = wpool.tile([P, n_mb], f32)
        dinv_colq = wpool.tile([P, n_mb], f32)
        corr4 = wpool.tile([1, 4, D], f32)
        corr = wpool.tile([1, D], f32)
        corr_hi = wpool.tile([1, D], bf16)
        corr_lo = wpool.tile([1, D], bf16)

        nc.vector.memset(acc_v[:], 0.0)

        # issue all loads up-front on the sync ring; interleave pair loads
        # with e3 stripes so DVE's stripes arrive spread through the window
        dmas = []
        for k in range(len(pairs)):
            dmas.append(("p", k))
        for idx, i in enumerate(e3_order):
            dmas.append(("s", idx))
        # round-robin: pair, e3, pair, e3 ... then leftovers in order
        order, a, b = [], 0, 0
        for t in range(len(dmas)):
            if t % 2 == 0 and a < len(pairs):
                order.append(("p", a)); a += 1
            elif b < len(e3_order):
                order.append(("s", b)); b += 1
            elif a < len(pairs):
                order.append(("p", a)); a += 1
        for kind, idx in order:
            if kind == "p":
                nc.sync.dma_start(
                    pair_tiles[idx][:],
                    A4P[idx * P:(idx + 1) * P, :].rearrange(
                        "p (two n) -> p two n", two=2))
            else:
                nc.sync.dma_start(e3_tiles[idx][:],
                                  A8[idx * P:(idx + 1) * P, :])

        if mode == "load":
            return
        if not h_after:
            _h_phase(nc, tc, mybir, n_mb, xta_sb, wb_sb, h_all)
        # ---- Phase 1: degree pass over arriving stripes ----
        with tc.tile_pool(name="dpsum", bufs=1, space="PSUM") as dpsum:
            d_acc = [dpsum.tile([D, CHUNK], f32, name=f"d_acc{c}",
                                tag=f"d_acc{c}") for c in range(n_ch)]
            DR = mybir.MatmulPerfMode.DoubleRow
            first_pe = True
            for k in range(len(pairs)):
                for c in range(n_ch):
                    nc.tensor.matmul(
                        d_acc[c][:], ones_e4[:],
                        pair_tiles[k][:, :, c * CHUNK:(c + 1) * CHUNK],
                        perf_mode=DR, start=first_pe, stop=False)
                first_pe = False
            for i in range(n_mb):
                if i in paired:
                    continue
                if i in dve_set:
                    nc.vector.tensor_tensor(
                        acc_v[:], acc_v[:], stripe_ap(i), add)
                else:
                    for c in range(n_ch):
                        nc.tensor.matmul(
                            d_acc[c][:], ones_bf[:],
                            stripe_ap(i)[:, c * CHUNK:(c + 1) * CHUNK],
                            start=first_pe, stop=False)
                    first_pe = False
            # fold the DVE accumulator into PSUM
            for c in range(n_ch):
                nc.tensor.matmul(
                    d_acc[c][:], ones_bf[:],
                    acc_v[:, c * CHUNK:(c + 1) * CHUNK],
                    start=False, stop=True)
            # raw colsum row 0 -> SBUF -> DRAM per chunk (feeds the
            # compact dinv path; per-chunk so the bounce overlaps the folds)
            for c in range(n_ch):
                nc.scalar.copy(d_row[0:1, c * CHUNK:(c + 1) * CHUNK],
                               d_acc[c][0:1, :])
                if chunk_bounce:
                    nc.sync.dma_start(OT[0:1, c * CHUNK:(c + 1) * CHUNK],
                                      d_row[0:1, c * CHUNK:(c + 1) * CHUNK])
            # replicated path: dinv_rep = (0.25*colsum + 2049)^-1/2
            # (sqrt now from PSUM; in-place reciprocal later, off the
            # critical path)
            for c in range(n_ch):
                nc.scalar.activation(
                    dinv_rep[:, c * CHUNK:(c + 1) * CHUNK], d_acc[c][:],
                    Sqrt, bias=bias_rep[0:D, :], scale=0.25)

        if h_after:
            _h_phase(nc, tc, mybir, n_mb, xta_sb, wb_sb, h_all)

        # ---- Phase 2: compact dinv via DRAM bounce + PE transpose ----
        # bounce raw colsum through row 0 of OT (overwritten by the final
        # store); gives [32,128] layout for a single PE transpose.
        if not chunk_bounce:
            nc.sync.dma_start(OT[0:1, :], d_row[0:1, :])
        nc.sync.dma_start(
            dcol32[:, :], OT[0, :].rearrange("(q p) -> q p", q=n_mb))
        with tc.tile_pool(name="tpsum", bufs=1, space="PSUM") as tpsum:
            tp = tpsum.tile([P, n_mb], f32)
            nc.tensor.transpose(tp[:], dcol32[:, :], eye_sb[:])
            nc.vector.tensor_copy(dcol[:], tp[:])
        # dinv_colq = (4*colsum + 32784)^-1/2 = 0.25 * dinv  (folds the /4
        # of the a8 encoding into G)
        nc.scalar.activation(dsq_col[:], dcol[:], Sqrt,
                             bias=bias_col[:], scale=4.0)
        nc.vector.reciprocal(dinv_colq[:], dsq_col[:])

        # ---- Phase 3: G = dinv/4 * H (bf16), centering correction ----
        nc.vector.tensor_tensor(
            g_q[:], h_all[:],
            dinv_colq[:, :, None].to_broadcast((P, n_mb, D)), mult)
        with tc.tile_pool(name="cpsum", bufs=1, space="PSUM") as cpsum:
            cs = [cpsum.tile([1, CHUNK], f32, name=f"cs{t}", tag=f"cs{t}")
                  for t in range(4)]
            for t in range(4):
                nc.tensor.matmul(
                    cs[t][:], ones_bf[:, 0:1], g_q[:, t * 8:(t + 1) * 8, :],
                    start=True, stop=True)
            for t in range(4):
                nc.vector.tensor_reduce(
                    corr4[0:1, t, :],
                    cs[t].rearrange("p (s d) -> p d s", s=8), AX, add)
        nc.vector.tensor_reduce(
            corr[0:1, :], corr4.rearrange("p t d -> p d t"), AX, add)
        nc.vector.tensor_copy(corr_hi[:], corr[:])
        nc.vector.tensor_tensor(corr_lo[:], corr[:], corr_hi[:],
                                mybir.AluOpType.subtract)
        # finish the replicated dinv (overlaps matmul below)
        nc.vector.reciprocal(dinv_rep[:], dinv_rep[:])

        if mode == "nomm":
            return
        # ---- Phase 4: aggregation, chunk-outer ----
        with tc.tile_pool(name="opsum", bufs=1, space="PSUM") as opsum, \
             tc.tile_pool(name="obuf", bufs=1) as obuf:
            for c in range(n_ch):
                sl = slice(c * CHUNK, (c + 1) * CHUNK)
                o_acc = opsum.tile([D, CHUNK], f32, name=f"o_acc{c}",
                                   tag=f"o_acc{c % 4}")
                for i in range(n_mb):
                    nc.tensor.matmul(
                        o_acc[:], g_q[:, i, :], stripe_ap(i)[:, sl],
                        start=(i == 0), stop=False)
                per_bank = CHUNK // P
                for jj in range(per_bank):
                    j = c * per_bank + jj
                    nc.tensor.matmul(
                        o_acc[:, jj * P:(jj + 1) * P], g_q[:, j, :],
                        eye4_sb[:], start=False, stop=False)
                nc.tensor.matmul(
                    o_acc[:], corr_hi[:], ones2[:],
                    start=False, stop=False)
                nc.tensor.matmul(
                    o_acc[:], corr_lo[:], ones2[:],
                    start=False, stop=True)
                ob = obuf.tile([D, CHUNK], f32, name=f"ob{c}",
                               tag=f"ob{c % 4}")
                nc.vector.tensor_tensor(ob[:], o_acc[:], dinv_rep[:, sl],
                                        mult)
                nc.sync.dma_start(OT[:, sl], ob[:])


def _get_program(key):
    if key not in _prog_cache:
        n, reps = key
        _prog_cache[key] = build_program(n=n, reps=reps)
    return _prog_cache[key]


def make_in_maps(X, A, W, b, dve_every=7, dr_pairs=7, **_ignored):
    import ml_dtypes
    n = A.shape[1]
    n_mb = n // P
    e3 = ml_dtypes.float8_e3m4
    e4 = ml_dtypes.float8_e4m3
    bf = ml_dtypes.bfloat16
    dve_set = set(range(1, n_mb, dve_every)) if dve_every else set()
    pe_list = [i for i in range(n_mb) if i not in dve_set]
    pair_stripes = pe_list[:2 * dr_pairs]
    pairs = [(pair_stripes[2 * k], pair_stripes[2 * k + 1])
             for k in range(dr_pairs)]
    in_pair = set(pair_stripes)
    e3_order = [i for i in range(n_mb) if i not in in_pair]
    eye = np.eye(32, dtype=np.float32)
    eye4 = (4.0 * np.eye(P, dtype=np.float32)).astype(bf)
    Wb = np.concatenate([W.astype(np.float32),
                         b.astype(np.float32)[None, :]], axis=0).astype(bf)
    in_maps = []
    for i in range(X.shape[0]):
        AT = np.ascontiguousarray(np.asarray(A[i]).T, dtype=np.float32)
        ATc = (AT - 0.5) * 4.0
        A8 = np.concatenate(
            [ATc[j * P:(j + 1) * P, :] for j in e3_order],
            axis=0).astype(e3) if e3_order else np.zeros((0, n), e3)
        m = {"A8": A8}
        if pairs:
            a4p = np.empty((dr_pairs * P, 2, n), np.float32)
            for k, (s0, s1) in enumerate(pairs):
                a4p[k * P:(k + 1) * P, 0, :] = ATc[s0 * P:(s0 + 1) * P, :]
                a4p[k * P:(k + 1) * P, 1, :] = ATc[s1 * P:(s1 + 1) * P, :]
            m["A4P"] = a4p.reshape(dr_pairs * P, 2 * n).astype(e4)
        XTa = np.concatenate(
            [np.ascontiguousarray(np.asarray(X[i]).T),
             np.ones((1, n), np.float32)], axis=0).astype(bf)
        m.update({"XTA": XTa, "WB": Wb, "EYE": eye, "EYE4": eye4})
        in_maps.append(m)
    return in_maps


def kernel(X, A, W, b, reps=1, **_ignored):
    from concourse.bass_utils import run_bass_kernel_spmd

    X = np.asarray(X, dtype=np.float32)
    A = np.asarray(A, dtype=np.float32)
    W = np.asarray(W, dtype=np.float32)
    b = np.asarray(b, dtype=np.float32)
    n_b, n, _ = A.shape
    nc = _get_program((n, reps))
    in_maps = make_in_maps(X, A, W, b)
    res = run_bass_kernel_spmd(nc, in_maps, list(range(n_b)))
    out = np.stack([res.results[i]["OT"].T for i in range(n_b)])
    return np.ascontiguousarray(out)


def _h_phase(nc, tc, mybir, n_mb, xta_sb, wb_sb, h_all):
    f32 = mybir.dt.float32
    with tc.tile_pool(name="hpsum", bufs=2, space="PSUM") as hpsum:
        for blk in range(n_mb // 8):
            hp = hpsum.tile([P, 8 * D], f32, name=f"hp{blk}", tag="hp")
            for jj in range(8):
                j = blk * 8 + jj
                nc.tensor.matmul(
                    hp[:, jj * D:(jj + 1) * D],
                    xta_sb[:, j * P:(j + 1) * P], wb_sb[:],
                    start=True, stop=True)
            nc.scalar.copy(h_all[:, blk * 8:(blk + 1) * 8, :], hp[:])
